# revision 25
# baseline (speedup 1.0000x reference)
# Trainium2 Bass kernel for nn_CN_MLP_71631464563230 (moe_routing).
#
# Math: the reference is
#   mo = x @ W.T + b;  w = softmax(mo @ attn);  out = sigmoid(w . (mo @ V.T) + cla_b)
# with V[t,h] = sum_k CM[t,h,k] cla_w[k]. Both pre-softmax quantities are
# LINEAR in mo, and mo is affine in x, so by associativity
#   a  = mo @ attn = x @ (W.T @ attn) + (b @ attn)
#   s  = mo @ V.T  = x @ (W.T @ V.T)  + (V @ b)
# The parameter-only folds G = [W.T attn | W.T V.T] (D x 2T), ca, cs are
# precomputed on the host at pack time (constant folding through linear
# layers, like BN-into-conv). The device computes asT = G.T x.T per core
# (fp8 DoubleRow, 16x fewer MACs than materializing mo) and the nonlinear
# epilogue:  out = sigmoid( (sum_t e^{a} s)/(sum_t e^{a}) + cla_b ).
# CM / mlp_w / attn never need to be DMA'd: per-core traffic drops from
# 16.4MB to 5.4MB, and the old ~40us DVE V-fold disappears entirely.
#
# Sharding: batch 8x data-parallel (1024 rows/core); G + consts replicated
# (G is 320KB fp8). G is scaled by 64 host-side to clear the e4m3 denormal
# floor (G elems ~ N(0, 1/5000)); the scale is inverted in the ACT scale
# operand of the two Exp ops, and cs is pre-multiplied by it. a lands in
# PSUM partitions 0:16 and s in 32:48 (DVE reads of PSUM must be
# 32-partition-quadrant aligned). Only the Exp ACT table is ever loaded
# (the sigmoid is exp + fast-reciprocal; ACT Sigmoid lives in a different
# table set and thrashes 1.3us loads between groups).
#
# DMA reality: each dma_start costs ~700ns of trigger time on its issuing
# engine and partition lines under ~4KB drop throughput, so x streams in 9
# big transfers (5KB lines) on the sync queue while G + consts go on the
# scalar queue in parallel. The batch is split into 4 groups of 256 so
# each group's epilogue (8 cross-engine latency-dominated ops) overlaps
# the next group's stream; the last group's k-chunks taper (20/14/6) so
# little matmul work trails the final byte. Every group has its own SBUF
# tiles and PSUM banks — slices of shared tiles create false WAR deps
# that lockstep-serialize the group chains.

import os

import ml_dtypes
import numpy as np

import concourse.bass as bass
import concourse.mybir as mybir
import concourse.tile as tile
from concourse import bacc
from concourse.bass_utils import run_bass_kernel_spmd

B, D, H, T = 8192, 5000, 512, 16
NCORES = 8
BLOC = B // NCORES            # 1024 batch rows per core
KT = (D + 127) // 128         # 40 k-tiles over D (last padded)
KP = KT // 2                  # 20 DoubleRow k-pairs
GBS = [256, 256, 256, 256]    # batch rows per group
NG = len(GBS)
GOFF = [sum(GBS[:g]) for g in range(NG)]
XBASE = [KT * sum(GBS[:g]) for g in range(NG)]   # flat x offset of group g
CHUNKS = [[(0, 20), (20, 40)],
          [(0, 20), (20, 40)],
          [(0, 20), (20, 40)],
          [(0, 20), (20, 34), (34, 40)]]
G_SCALE = 64.0
M2 = 64                       # PE out partitions: a at 0:16, s at 32:48

F32 = mybir.dt.float32
BF = mybir.dt.bfloat16
F8 = mybir.dt.float8e4
NP_F8 = mybir.dt.np(F8)       # ml_dtypes.float8_e4m3 (TRN semantics, max 240)
DR = mybir.MatmulPerfMode.DoubleRow
AF = mybir.ActivationFunctionType

LAST_RESULTS = None


def _build_nc():
    nc = bacc.Bacc("TRN2", target_bir_lowering=False)

    xT = nc.dram_tensor("xT", [128, KT * BLOC], F8, kind="ExternalInput").ap()
    gT = nc.dram_tensor("gT", [128, KT * M2], F8, kind="ExternalInput").ap()
    ca_d = nc.dram_tensor("ca", [T, 1], F32, kind="ExternalInput").ap()
    cs_d = nc.dram_tensor("csp", [T, 1], F32, kind="ExternalInput").ap()
    clabh = nc.dram_tensor("clabh", [1, 1], F32, kind="ExternalInput").ap()
    out_d = nc.dram_tensor("out", [1, BLOC], F32, kind="ExternalOutput").ap()

    with tile.TileContext(nc) as tc:
        import contextlib

        ctx = contextlib.ExitStack()
        with ctx:
            sg = ctx.enter_context(tc.tile_pool(name="sg", bufs=1))
            pp = ctx.enter_context(tc.tile_pool(name="pp", bufs=1, space="PSUM"))

            # ---- tiles ----------------------------------------------------
            G_sb = sg.tile([128, KT, M2], F8, tag="G_sb")
            xch = {}
            for g in range(NG):
                for (k0, k1) in CHUNKS[g]:
                    xch[g, k0] = sg.tile([128, k1 - k0, GBS[g]], F8,
                                         tag=f"x{g}_{k0}", name=f"x{g}_{k0}")
            ca_sb = sg.tile([T, 1], F32, tag="ca_sb")
            cs_sb = sg.tile([T, 1], F32, tag="cs_sb")
            clabh_sb = sg.tile([1, 1], F32, tag="clabh_sb")
            ones16 = sg.tile([T, 1], BF, tag="ones16")
            warm1 = sg.tile([1, 1], F32, tag="warm1")
            EP_sb = [sg.tile([T, 2, GBS[g]], BF, tag=f"EP{g}", name=f"EP{g}")
                     for g in range(NG)]
            rden = [sg.tile([1, GBS[g]], F32, tag=f"rden{g}", name=f"rden{g}")
                    for g in range(NG)]
            lg = [sg.tile([1, GBS[g]], F32, tag=f"lg{g}", name=f"lg{g}")
                  for g in range(NG)]
            th = [sg.tile([1, GBS[g]], F32, tag=f"th{g}", name=f"th{g}")
                  for g in range(NG)]
            orow = [sg.tile([1, GBS[g]], F32, tag=f"orow{g}", name=f"orow{g}")
                    for g in range(NG)]

            mm_ps = [pp.tile([M2, GBS[g]], F32, tag=f"p{g}", name=f"mm{g}")
                     for g in range(NG)]

            # ---- x stream split across BOTH hwdge queues (sync + scalar,
            # alternating chunks): triggers pipeline in parallel and the
            # stream saturates ~420GB/s sooner. Side effect that MEASURES
            # BETTER than it sounds: the scalar engine sits in trigger
            # sem-reuse waits until the stream is nearly done, so the four
            # epilogue chains all become ready together and then pipeline
            # back-to-back with zero inter-op waits — denser than the
            # "prompt" ordering, where each chain's tanh relays in front
            # of the next group's E on the in-order ACT engine ------------
            nc.scalar.dma_start(
                out=G_sb.rearrange("p k m -> p (k m)"), in_=gT)
            ci = 0
            for g in range(NG):
                for (k0, k1) in CHUNKS[g]:
                    lo = XBASE[g] + k0 * GBS[g]
                    eng = nc.sync if ci % 2 == 0 else nc.scalar
                    eng.dma_start(
                        out=xch[g, k0].rearrange("p k b -> p (k b)"),
                        in_=xT[:, lo:lo + (k1 - k0) * GBS[g]])
                    ci += 1

            # ---- consts on the scalar queue; Exp table preload ------------
            nc.scalar.dma_start(out=ca_sb, in_=ca_d)
            nc.scalar.dma_start(out=cs_sb, in_=cs_d)
            nc.scalar.dma_start(out=clabh_sb, in_=clabh)
            nc.gpsimd.memset(ones16, 1.0)
            nc.scalar.activation(warm1, clabh_sb, AF.Exp)

            # ---- per-group matmuls and 3-stage epilogue -------------------
            def mms(g):
                for (k0, k1) in CHUNKS[g]:
                    xt = xch[g, k0]
                    for kp in range(k0 // 2, k1 // 2):
                        lk = 2 * kp - k0
                        nc.tensor.matmul(
                            mm_ps[g],
                            lhsT=G_sb[:, 2 * kp:2 * kp + 2, :],
                            rhs=xt[:, lk:lk + 2, :],
                            start=(kp == 0), stop=(kp == KP - 1),
                            perf_mode=DR)

            def stage_a(g):
                # E = exp(a/G_SCALE + ca);  P = (s + G_SCALE*cs) * E;
                # den|num via ones16 row-sum matmuls (fused for the small
                # last group; split in two for the big ones — a PSUM bank
                # holds 512 f32 per partition)
                gb = GBS[g]
                nc.scalar.activation(EP_sb[g][:, 0, :], mm_ps[g][0:T, :],
                                     AF.Exp, bias=ca_sb, scale=1.0 / G_SCALE)
                nc.vector.scalar_tensor_tensor(
                    out=EP_sb[g][:, 1, :], in0=mm_ps[g][32:48, :],
                    scalar=cs_sb, in1=EP_sb[g][:, 0, :],
                    op0=mybir.AluOpType.add, op1=mybir.AluOpType.mult)
                dn_ps = pp.tile([1, 2 * gb], F32, tag=f"p{4 + g}",
                                name=f"dn{g}")
                nc.tensor.matmul(
                    dn_ps, lhsT=ones16,
                    rhs=EP_sb[g].rearrange("t a b -> t (a b)"),
                    start=True, stop=True)
                return dn_ps[:, 0:gb], dn_ps[:, gb:2 * gb]

            def stage_b(g, dn):
                den_ps, num_ps = dn
                nc.vector.reciprocal_approx_fast(out=rden[g], in_=den_ps)
                nc.vector.tensor_mul(lg[g], num_ps, rden[g])

            def stage_c(g):
                # sigmoid(z) = 0.5*tanh(z/2) + 0.5, z = num/den/G_SCALE+cla_b
                # (Tanh shares the Exp ACT table set: no table thrash).
                nc.scalar.activation(th[g], lg[g], AF.Tanh,
                                     bias=clabh_sb, scale=0.5 / G_SCALE)
                # 0.5*tanh + 0.5 as an ACT Copy (Copy is in every table
                # set): stays on the same engine as tanh (no hop) and
                # rebalances the DVE-limited dense chain phase
                nc.scalar.activation(orow[g], th[g], AF.Copy,
                                     bias=0.5, scale=0.5)
                nc.sync.dma_start(
                    out=out_d[:, GOFF[g]:GOFF[g] + GBS[g]], in_=orow[g])

            dn = {}
            for line in range(NG + 3):
                if line < NG:
                    mms(line)
                if 0 <= line - 1 < NG:
                    dn[line - 1] = stage_a(line - 1)
                if 0 <= line - 2 < NG:
                    stage_b(line - 2, dn[line - 2])
                if 0 <= line - 3 < NG:
                    stage_c(line - 3)

    nc.finalize()
    return nc


_NC_CACHE = None


def _pack_inputs(data_input, mlp_w, mlp_b, CM, attn, cla_w, cla_b):
    x = np.asarray(data_input, dtype=np.float32)
    W = np.asarray(mlp_w, dtype=np.float32)
    b = np.asarray(mlp_b, dtype=np.float32)
    CM = np.asarray(CM, dtype=np.float32)
    attn = np.asarray(attn, dtype=np.float32)
    cla_w = np.asarray(cla_w, dtype=np.float32).reshape(H)
    cla_b = np.asarray(cla_b, dtype=np.float32).reshape(1, 1)

    # Parameter folds (host, O(D*H) — data-independent)
    V = CM @ cla_w                       # [T, H]
    Ga = W.T @ attn                      # [D, T]
    Gs = W.T @ V.T                       # [D, T]
    ca = (b @ attn).reshape(T, 1)
    csp = (G_SCALE * (V @ b)).reshape(T, 1)

    DP = KT * 128
    # x: [B, D] -> per core [128, (g kt j)] fp8, group-major, sizes GBS
    xp = np.zeros((B, DP), dtype=np.float32)
    xp[:, :D] = np.clip(x, -240, 240)
    xp = xp.reshape(NCORES, BLOC, KT, 128)
    slabs = []
    for g in range(NG):
        blk = (xp[:, GOFF[g]:GOFF[g] + GBS[g]]      # [core, gb, KT, 128]
               .transpose(0, 3, 2, 1)               # [core, 128, KT, gb]
               .reshape(NCORES, 128, KT * GBS[g]))
        slabs.append(blk)
    xpk = np.concatenate(slabs, axis=2).astype(NP_F8)
    # G: [D, 2T] -> [128, (kt m)] fp8, scaled, quadrant-padded
    gp = np.zeros((DP, M2), dtype=np.float32)
    gp[:D, 0:T] = np.clip(Ga * G_SCALE, -240, 240)
    gp[:D, 32:32 + T] = np.clip(Gs * G_SCALE, -240, 240)
    gp = (gp.reshape(KT, 128, M2).transpose(1, 0, 2)
            .reshape(128, KT * M2).astype(NP_F8))

    shared = {"gT": gp, "ca": np.ascontiguousarray(ca),
              "csp": np.ascontiguousarray(csp),
              "clabh": np.ascontiguousarray(0.5 * cla_b)}
    return [
        {"xT": np.ascontiguousarray(xpk[i]), **shared}
        for i in range(NCORES)
    ]


def kernel(data_input, mlp_w, mlp_b, CM, attn, cla_w, cla_b):
    global LAST_RESULTS, _NC_CACHE

    in_maps = _pack_inputs(data_input, mlp_w, mlp_b, CM, attn, cla_w, cla_b)

    if _NC_CACHE is None:
        _NC_CACHE = _build_nc()

    trace = bool(int(os.environ.get("KERNEL_TRACE", "0")))
    res = run_bass_kernel_spmd(
        _NC_CACHE, in_maps, core_ids=list(range(NCORES)), trace=trace,
        trace_cores=[0] if trace else None,
    )
    LAST_RESULTS = res

    full = np.empty(B, dtype=np.float32)
    for i in range(NCORES):
        full[i * BLOC:(i + 1) * BLOC] = res.results[i]["out"].reshape(BLOC)
    return full


# revision 26
# speedup vs baseline: 1.1195x; 1.1195x over previous
# Trainium2 Bass kernel for nn_CN_MLP_71631464563230 (moe_routing).
#
# Math: the reference is
#   mo = x @ W.T + b;  w = softmax(mo @ attn);  out = sigmoid(w . (mo @ V.T) + cla_b)
# with V[t,h] = sum_k CM[t,h,k] cla_w[k]. Both pre-softmax quantities are
# LINEAR in mo, and mo is affine in x, so by associativity
#   a  = mo @ attn = x @ (W.T @ attn) + (b @ attn)
#   s  = mo @ V.T  = x @ (W.T @ V.T)  + (V @ b)
# The parameter-only folds G = [W.T attn | W.T V.T] (D x 2T), ca, cs are
# precomputed on the host at pack time (constant folding through linear
# layers, like BN-into-conv). The device computes asT = G.T x.T per core
# (fp8 DoubleRow, 16x fewer MACs than materializing mo) and the nonlinear
# epilogue:  out = sigmoid( (sum_t e^{a} s)/(sum_t e^{a}) + cla_b ).
# CM / mlp_w / attn never need to be DMA'd: per-core traffic drops from
# 16.4MB to 5.4MB, and the old ~40us DVE V-fold disappears entirely.
#
# Sharding: batch 8x data-parallel (1024 rows/core); G + consts replicated
# (G is 320KB fp8). G is scaled by 64 host-side to clear the e4m3 denormal
# floor (G elems ~ N(0, 1/5000)); the scale is inverted in the ACT scale
# operand of the two Exp ops, and cs is pre-multiplied by it. a lands in
# PSUM partitions 0:16 and s in 32:48 (DVE reads of PSUM must be
# 32-partition-quadrant aligned). Only the Exp ACT table is ever loaded
# (the sigmoid is exp + fast-reciprocal; ACT Sigmoid lives in a different
# table set and thrashes 1.3us loads between groups).
#
# DMA reality: each dma_start costs ~700ns of trigger time on its issuing
# engine and partition lines under ~4KB drop throughput, so x streams in 9
# big transfers (5KB lines) on the sync queue while G + consts go on the
# scalar queue in parallel. The batch is split into 4 groups of 256 so
# each group's epilogue (8 cross-engine latency-dominated ops) overlaps
# the next group's stream; the last group's k-chunks taper (20/14/6) so
# little matmul work trails the final byte. Every group has its own SBUF
# tiles and PSUM banks — slices of shared tiles create false WAR deps
# that lockstep-serialize the group chains.

import os

import ml_dtypes
import numpy as np

import concourse.bass as bass
import concourse.mybir as mybir
import concourse.tile as tile
from concourse import bacc
from concourse.bass_utils import run_bass_kernel_spmd

B, D, H, T = 8192, 5000, 512, 16
NCORES = 8
BLOC = B // NCORES            # 1024 batch rows per core
KT = (D + 127) // 128         # 40 k-tiles over D (last padded)
KP = KT // 2                  # 20 DoubleRow k-pairs
GBS = [256, 256, 256, 256]    # batch rows per group
NG = len(GBS)
GOFF = [sum(GBS[:g]) for g in range(NG)]
XBASE = [KT * sum(GBS[:g]) for g in range(NG)]   # flat x offset of group g
CHUNKS = [[(0, 20), (20, 40)],
          [(0, 20), (20, 40)],
          [(0, 20), (20, 40)],
          [(0, 20), (20, 34), (34, 40)]]
G_SCALE = 64.0
M2 = 64                       # PE out partitions: a at 0:16, s at 32:48

F32 = mybir.dt.float32
BF = mybir.dt.bfloat16
F8 = mybir.dt.float8e4
NP_F8 = mybir.dt.np(F8)       # ml_dtypes.float8_e4m3 (TRN semantics, max 240)
DR = mybir.MatmulPerfMode.DoubleRow
AF = mybir.ActivationFunctionType

LAST_RESULTS = None


def _build_nc():
    nc = bacc.Bacc("TRN2", target_bir_lowering=False)

    xT = nc.dram_tensor("xT", [128, KT * BLOC], F8, kind="ExternalInput").ap()
    gT = nc.dram_tensor("gT", [128, KT * M2], F8, kind="ExternalInput").ap()
    ca_d = nc.dram_tensor("ca", [T, 1], F32, kind="ExternalInput").ap()
    cs_d = nc.dram_tensor("csp", [T, 1], F32, kind="ExternalInput").ap()
    clabh = nc.dram_tensor("clabh", [1, 1], F32, kind="ExternalInput").ap()
    out_d = nc.dram_tensor("out", [1, BLOC], F32, kind="ExternalOutput").ap()

    with tile.TileContext(nc) as tc:
        import contextlib

        ctx = contextlib.ExitStack()
        with ctx:
            sg = ctx.enter_context(tc.tile_pool(name="sg", bufs=1))
            pp = ctx.enter_context(tc.tile_pool(name="pp", bufs=1, space="PSUM"))

            # ---- tiles ----------------------------------------------------
            G_sb = sg.tile([128, KT, M2], F8, tag="G_sb")
            xch = {}
            for g in range(NG):
                for (k0, k1) in CHUNKS[g]:
                    xch[g, k0] = sg.tile([128, k1 - k0, GBS[g]], F8,
                                         tag=f"x{g}_{k0}", name=f"x{g}_{k0}")
            ca_sb = sg.tile([T, 1], F32, tag="ca_sb")
            cs_sb = sg.tile([T, 1], F32, tag="cs_sb")
            clabh_sb = sg.tile([1, 1], F32, tag="clabh_sb")
            ones16 = sg.tile([T, 1], BF, tag="ones16")
            warm1 = sg.tile([1, 1], F32, tag="warm1")
            EP_sb = [sg.tile([T, 2, GBS[g]], BF, tag=f"EP{g}", name=f"EP{g}")
                     for g in range(NG)]
            rden = [sg.tile([1, GBS[g]], F32, tag=f"rden{g}", name=f"rden{g}")
                    for g in range(NG)]
            lg = [sg.tile([1, GBS[g]], F32, tag=f"lg{g}", name=f"lg{g}")
                  for g in range(NG)]
            th = [sg.tile([1, GBS[g]], F32, tag=f"th{g}", name=f"th{g}")
                  for g in range(NG)]
            orow = [sg.tile([1, GBS[g]], F32, tag=f"orow{g}", name=f"orow{g}")
                    for g in range(NG)]

            mm_ps = [pp.tile([M2, GBS[g]], F32, tag=f"p{g}", name=f"mm{g}")
                     for g in range(NG)]

            # ---- x stream split across BOTH hwdge queues (sync + scalar,
            # alternating chunks): triggers pipeline in parallel and the
            # stream saturates ~420GB/s sooner. Side effect that MEASURES
            # BETTER than it sounds: the scalar engine sits in trigger
            # sem-reuse waits until the stream is nearly done, so the four
            # epilogue chains all become ready together and then pipeline
            # back-to-back with zero inter-op waits — denser than the
            # "prompt" ordering, where each chain's tanh relays in front
            # of the next group's E on the in-order ACT engine ------------
            nc.scalar.dma_start(
                out=G_sb.rearrange("p k m -> p (k m)"), in_=gT)
            ci = 0
            for g in range(NG):
                for (k0, k1) in CHUNKS[g]:
                    lo = XBASE[g] + k0 * GBS[g]
                    eng = nc.sync if ci % 2 == 0 else nc.scalar
                    eng.dma_start(
                        out=xch[g, k0].rearrange("p k b -> p (k b)"),
                        in_=xT[:, lo:lo + (k1 - k0) * GBS[g]])
                    ci += 1

            # ---- consts on the scalar queue; Exp table preload ------------
            nc.scalar.dma_start(out=ca_sb, in_=ca_d)
            nc.scalar.dma_start(out=cs_sb, in_=cs_d)
            nc.scalar.dma_start(out=clabh_sb, in_=clabh)
            nc.gpsimd.memset(ones16, 1.0)
            nc.scalar.activation(warm1, clabh_sb, AF.Exp)

            # ---- per-group matmuls and 3-stage epilogue -------------------
            def mms(g):
                for (k0, k1) in CHUNKS[g]:
                    xt = xch[g, k0]
                    for kp in range(k0 // 2, k1 // 2):
                        lk = 2 * kp - k0
                        nc.tensor.matmul(
                            mm_ps[g],
                            lhsT=G_sb[:, 2 * kp:2 * kp + 2, :],
                            rhs=xt[:, lk:lk + 2, :],
                            start=(kp == 0), stop=(kp == KP - 1),
                            perf_mode=DR)

            def stage_a(g):
                # E = exp(a/G_SCALE + ca);  P = (s + G_SCALE*cs) * E;
                # den|num via ones16 row-sum matmuls (fused for the small
                # last group; split in two for the big ones — a PSUM bank
                # holds 512 f32 per partition)
                gb = GBS[g]
                nc.scalar.activation(EP_sb[g][:, 0, :], mm_ps[g][0:T, :],
                                     AF.Exp, bias=ca_sb, scale=1.0 / G_SCALE)
                nc.vector.scalar_tensor_tensor(
                    out=EP_sb[g][:, 1, :], in0=mm_ps[g][32:48, :],
                    scalar=cs_sb, in1=EP_sb[g][:, 0, :],
                    op0=mybir.AluOpType.add, op1=mybir.AluOpType.mult)
                dn_ps = pp.tile([1, 2 * gb], F32, tag=f"p{4 + g}",
                                name=f"dn{g}")
                nc.tensor.matmul(
                    dn_ps, lhsT=ones16,
                    rhs=EP_sb[g].rearrange("t a b -> t (a b)"),
                    start=True, stop=True)
                return dn_ps[:, 0:gb], dn_ps[:, gb:2 * gb]

            def stage_b(g, dn):
                den_ps, num_ps = dn
                nc.vector.reciprocal_approx_fast(out=rden[g], in_=den_ps)
                nc.vector.tensor_mul(lg[g], num_ps, rden[g])

            def stage_c(g):
                # sigmoid(z) = 0.5*tanh(z/2) + 0.5, z = num/den/G_SCALE+cla_b
                # (Tanh shares the Exp ACT table set: no table thrash).
                nc.scalar.activation(th[g], lg[g], AF.Tanh,
                                     bias=clabh_sb, scale=0.5 / G_SCALE)
                nc.vector.tensor_scalar(
                    out=orow[g], in0=th[g], scalar1=0.5, scalar2=0.5,
                    op0=mybir.AluOpType.mult, op1=mybir.AluOpType.add)
                nc.sync.dma_start(
                    out=out_d[:, GOFF[g]:GOFF[g] + GBS[g]], in_=orow[g])

            dn = {}
            for line in range(NG + 3):
                if line < NG:
                    mms(line)
                if 0 <= line - 1 < NG:
                    dn[line - 1] = stage_a(line - 1)
                if 0 <= line - 2 < NG:
                    stage_b(line - 2, dn[line - 2])
                if 0 <= line - 3 < NG:
                    stage_c(line - 3)

    nc.finalize()
    return nc


_NC_CACHE = None


def _pack_inputs(data_input, mlp_w, mlp_b, CM, attn, cla_w, cla_b):
    x = np.asarray(data_input, dtype=np.float32)
    W = np.asarray(mlp_w, dtype=np.float32)
    b = np.asarray(mlp_b, dtype=np.float32)
    CM = np.asarray(CM, dtype=np.float32)
    attn = np.asarray(attn, dtype=np.float32)
    cla_w = np.asarray(cla_w, dtype=np.float32).reshape(H)
    cla_b = np.asarray(cla_b, dtype=np.float32).reshape(1, 1)

    # Parameter folds (host, O(D*H) — data-independent)
    V = CM @ cla_w                       # [T, H]
    Ga = W.T @ attn                      # [D, T]
    Gs = W.T @ V.T                       # [D, T]
    ca = (b @ attn).reshape(T, 1)
    csp = (G_SCALE * (V @ b)).reshape(T, 1)

    DP = KT * 128
    # x: [B, D] -> per core [128, (g kt j)] fp8, group-major, sizes GBS
    xp = np.zeros((B, DP), dtype=np.float32)
    xp[:, :D] = np.clip(x, -240, 240)
    xp = xp.reshape(NCORES, BLOC, KT, 128)
    slabs = []
    for g in range(NG):
        blk = (xp[:, GOFF[g]:GOFF[g] + GBS[g]]      # [core, gb, KT, 128]
               .transpose(0, 3, 2, 1)               # [core, 128, KT, gb]
               .reshape(NCORES, 128, KT * GBS[g]))
        slabs.append(blk)
    xpk = np.concatenate(slabs, axis=2).astype(NP_F8)
    # G: [D, 2T] -> [128, (kt m)] fp8, scaled, quadrant-padded
    gp = np.zeros((DP, M2), dtype=np.float32)
    gp[:D, 0:T] = np.clip(Ga * G_SCALE, -240, 240)
    gp[:D, 32:32 + T] = np.clip(Gs * G_SCALE, -240, 240)
    gp = (gp.reshape(KT, 128, M2).transpose(1, 0, 2)
            .reshape(128, KT * M2).astype(NP_F8))

    shared = {"gT": gp, "ca": np.ascontiguousarray(ca),
              "csp": np.ascontiguousarray(csp),
              "clabh": np.ascontiguousarray(0.5 * cla_b)}
    return [
        {"xT": np.ascontiguousarray(xpk[i]), **shared}
        for i in range(NCORES)
    ]


def kernel(data_input, mlp_w, mlp_b, CM, attn, cla_w, cla_b):
    global LAST_RESULTS, _NC_CACHE

    in_maps = _pack_inputs(data_input, mlp_w, mlp_b, CM, attn, cla_w, cla_b)

    if _NC_CACHE is None:
        _NC_CACHE = _build_nc()

    trace = bool(int(os.environ.get("KERNEL_TRACE", "0")))
    res = run_bass_kernel_spmd(
        _NC_CACHE, in_maps, core_ids=list(range(NCORES)), trace=trace,
        trace_cores=[0] if trace else None,
    )
    LAST_RESULTS = res

    full = np.empty(B, dtype=np.float32)
    for i in range(NCORES):
        full[i * BLOC:(i + 1) * BLOC] = res.results[i]["out"].reshape(BLOC)
    return full


# revision 27
# speedup vs baseline: 1.1790x; 1.0531x over previous
# Trainium2 Bass kernel for nn_CN_MLP_71631464563230 (moe_routing).
#
# Math: the reference is
#   mo = x @ W.T + b;  w = softmax(mo @ attn);  out = sigmoid(w . (mo @ V.T) + cla_b)
# with V[t,h] = sum_k CM[t,h,k] cla_w[k]. Both pre-softmax quantities are
# LINEAR in mo, and mo is affine in x, so by associativity
#   a  = mo @ attn = x @ (W.T @ attn) + (b @ attn)
#   s  = mo @ V.T  = x @ (W.T @ V.T)  + (V @ b)
# The parameter-only folds G = [W.T attn | W.T V.T] (D x 2T), ca, cs are
# precomputed on the host at pack time (constant folding through linear
# layers, like BN-into-conv). The device computes asT = G.T x.T per core
# (fp8 DoubleRow, 16x fewer MACs than materializing mo) and the nonlinear
# epilogue:  out = sigmoid( (sum_t e^{a} s)/(sum_t e^{a}) + cla_b ).
# CM / mlp_w / attn never need to be DMA'd: per-core traffic drops from
# 16.4MB to 5.4MB, and the old ~40us DVE V-fold disappears entirely.
#
# Sharding: batch 8x data-parallel (1024 rows/core); G + consts replicated
# (G is 320KB fp8). G is scaled by 64 host-side to clear the e4m3 denormal
# floor (G elems ~ N(0, 1/5000)); the scale is inverted in the ACT scale
# operand of the Exp/Tanh ops, and cs is pre-multiplied by it. a lands in
# PSUM partitions 0:16 and s in 32:48 (DVE reads of PSUM must be
# 32-partition-quadrant aligned). One ACT table load total: the sigmoid is
# 0.5*tanh(z/2)+0.5 and Tanh shares the Exp table set (ACT Sigmoid lives
# in a different set and thrashes 1.3us loads between groups).
#
# DMA reality: each dma_start costs ~700ns of trigger time on its issuing
# engine and partition lines under ~4KB drop throughput, so x streams in 9
# big transfers (5KB lines) alternating across both hwdge queues
# (sync + scalar) with G + consts leading the scalar queue. The batch is
# split into 4 groups of 256; each group's epilogue (7 cross-engine
# latency-dominated ops) overlaps the next group's stream, and the last
# group's k-chunks taper (20/14/6) so little matmul work trails the final
# byte. Every group has its own SBUF tiles and PSUM banks — slices of
# shared tiles create false WAR deps that lockstep-serialize the chains.

import os

import ml_dtypes
import numpy as np

import concourse.bass as bass
import concourse.mybir as mybir
import concourse.tile as tile
from concourse import bacc
from concourse.bass_utils import run_bass_kernel_spmd

B, D, H, T = 8192, 5000, 512, 16
NCORES = 8
BLOC = B // NCORES            # 1024 batch rows per core
KT = (D + 127) // 128         # 40 k-tiles over D (last padded)
KP = KT // 2                  # 20 DoubleRow k-pairs
GBS = [256, 256, 256, 256]    # batch rows per group
NG = len(GBS)
GOFF = [sum(GBS[:g]) for g in range(NG)]
XBASE = [KT * sum(GBS[:g]) for g in range(NG)]   # flat x offset of group g
CHUNKS = [[(0, 20), (20, 40)],
          [(0, 20), (20, 40)],
          [(0, 20), (20, 40)],
          [(0, 20), (20, 34), (34, 40)]]
G_SCALE = 64.0
M2 = 64                       # PE out partitions: a at 0:16, s at 32:48

F32 = mybir.dt.float32
BF = mybir.dt.bfloat16
F8 = mybir.dt.float8e4
NP_F8 = mybir.dt.np(F8)       # ml_dtypes.float8_e4m3 (TRN semantics, max 240)
DR = mybir.MatmulPerfMode.DoubleRow
AF = mybir.ActivationFunctionType

LAST_RESULTS = None


def _build_nc():
    nc = bacc.Bacc("TRN2", target_bir_lowering=False)

    xT = nc.dram_tensor("xT", [128, KT * BLOC], F8, kind="ExternalInput").ap()
    gT = nc.dram_tensor("gT", [128, KT * M2], F8, kind="ExternalInput").ap()
    ca_d = nc.dram_tensor("ca", [T, 1], F32, kind="ExternalInput").ap()
    cs_d = nc.dram_tensor("csp", [T, 1], F32, kind="ExternalInput").ap()
    clabh = nc.dram_tensor("clabh", [1, 1], F32, kind="ExternalInput").ap()
    out_d = nc.dram_tensor("out", [1, BLOC], F32, kind="ExternalOutput").ap()

    with tile.TileContext(nc) as tc:
        import contextlib

        ctx = contextlib.ExitStack()
        with ctx:
            sg = ctx.enter_context(tc.tile_pool(name="sg", bufs=1))
            pp = ctx.enter_context(tc.tile_pool(name="pp", bufs=1, space="PSUM"))

            # ---- tiles ----------------------------------------------------
            G_sb = sg.tile([128, KT, M2], F8, tag="G_sb")
            xch = {}
            for g in range(NG):
                for (k0, k1) in CHUNKS[g]:
                    xch[g, k0] = sg.tile([128, k1 - k0, GBS[g]], F8,
                                         tag=f"x{g}_{k0}", name=f"x{g}_{k0}")
            ca_sb = sg.tile([T, 1], F32, tag="ca_sb")
            cs_sb = sg.tile([T, 1], F32, tag="cs_sb")
            clabh_sb = sg.tile([1, 1], F32, tag="clabh_sb")
            ones16 = sg.tile([T, 1], BF, tag="ones16")
            warm1 = sg.tile([1, 1], F32, tag="warm1")
            EP_sb = [sg.tile([T, 2, GBS[g]], BF, tag=f"EP{g}", name=f"EP{g}")
                     for g in range(NG)]
            rden = [sg.tile([1, GBS[g]], F32, tag=f"rden{g}", name=f"rden{g}")
                    for g in range(NG)]
            lg = [sg.tile([1, GBS[g]], F32, tag=f"lg{g}", name=f"lg{g}")
                  for g in range(NG)]
            th = [sg.tile([1, GBS[g]], F32, tag=f"th{g}", name=f"th{g}")
                  for g in range(NG)]
            orow = [sg.tile([1, GBS[g]], F32, tag=f"orow{g}", name=f"orow{g}")
                    for g in range(NG)]

            mm_ps = [pp.tile([M2, GBS[g]], F32, tag=f"p{g}", name=f"mm{g}")
                     for g in range(NG)]

            # ---- x stream split across BOTH hwdge queues (sync + scalar,
            # alternating chunks): triggers pipeline in parallel and the
            # stream saturates ~420GB/s sooner. Side effect that MEASURES
            # BETTER than it sounds: the scalar engine sits in trigger
            # sem-reuse waits until the stream is nearly done, so the four
            # epilogue chains all become ready together and then pipeline
            # back-to-back with zero inter-op waits — denser than the
            # "prompt" ordering, where each chain's tanh relays in front
            # of the next group's E on the in-order ACT engine ------------
            nc.scalar.dma_start(
                out=G_sb.rearrange("p k m -> p (k m)"), in_=gT)
            ci = 0
            for g in range(NG):
                for (k0, k1) in CHUNKS[g]:
                    lo = XBASE[g] + k0 * GBS[g]
                    eng = nc.sync if ci % 2 == 0 else nc.scalar
                    eng.dma_start(
                        out=xch[g, k0].rearrange("p k b -> p (k b)"),
                        in_=xT[:, lo:lo + (k1 - k0) * GBS[g]])
                    ci += 1

            # ---- consts on the scalar queue; Exp table preload ------------
            nc.scalar.dma_start(out=ca_sb, in_=ca_d)
            nc.scalar.dma_start(out=cs_sb, in_=cs_d)
            nc.scalar.dma_start(out=clabh_sb, in_=clabh)
            nc.gpsimd.memset(ones16, 1.0)
            nc.scalar.activation(warm1, clabh_sb, AF.Exp)

            # ---- per-group matmuls and 3-stage epilogue -------------------
            def mms(g):
                for (k0, k1) in CHUNKS[g]:
                    xt = xch[g, k0]
                    for kp in range(k0 // 2, k1 // 2):
                        lk = 2 * kp - k0
                        nc.tensor.matmul(
                            mm_ps[g],
                            lhsT=G_sb[:, 2 * kp:2 * kp + 2, :],
                            rhs=xt[:, lk:lk + 2, :],
                            start=(kp == 0), stop=(kp == KP - 1),
                            perf_mode=DR)

            def stage_a(g):
                # E = exp(a/G_SCALE + ca);  P = (s + G_SCALE*cs) * E;
                # den|num via ones16 row-sum matmuls (fused for the small
                # last group; split in two for the big ones — a PSUM bank
                # holds 512 f32 per partition)
                gb = GBS[g]
                nc.scalar.activation(EP_sb[g][:, 0, :], mm_ps[g][0:T, :],
                                     AF.Exp, bias=ca_sb, scale=1.0 / G_SCALE)
                nc.vector.scalar_tensor_tensor(
                    out=EP_sb[g][:, 1, :], in0=mm_ps[g][32:48, :],
                    scalar=cs_sb, in1=EP_sb[g][:, 0, :],
                    op0=mybir.AluOpType.add, op1=mybir.AluOpType.mult)
                dn_ps = pp.tile([1, 2 * gb], F32, tag=f"p{4 + g}",
                                name=f"dn{g}")
                nc.tensor.matmul(
                    dn_ps, lhsT=ones16,
                    rhs=EP_sb[g].rearrange("t a b -> t (a b)"),
                    start=True, stop=True)
                return dn_ps[:, 0:gb], dn_ps[:, gb:2 * gb]

            def stage_b(g, dn):
                den_ps, num_ps = dn
                nc.vector.reciprocal_approx_fast(out=rden[g], in_=den_ps)
                nc.vector.tensor_mul(lg[g], num_ps, rden[g])

            def stage_c(g):
                # sigmoid(z) = 0.5*tanh(z/2) + 0.5, z = num/den/G_SCALE+cla_b
                # (Tanh shares the Exp ACT table set: no table thrash).
                nc.scalar.activation(th[g], lg[g], AF.Tanh,
                                     bias=clabh_sb, scale=0.5 / G_SCALE)
                nc.vector.tensor_scalar(
                    out=orow[g], in0=th[g], scalar1=0.5, scalar2=0.5,
                    op0=mybir.AluOpType.mult, op1=mybir.AluOpType.add)
                nc.sync.dma_start(
                    out=out_d[:, GOFF[g]:GOFF[g] + GBS[g]], in_=orow[g])

            dn = {}
            for line in range(NG + 3):
                if line < NG:
                    mms(line)
                if 0 <= line - 1 < NG:
                    dn[line - 1] = stage_a(line - 1)
                if 0 <= line - 2 < NG:
                    stage_b(line - 2, dn[line - 2])
                if 0 <= line - 3 < NG:
                    stage_c(line - 3)

    nc.finalize()
    return nc


_NC_CACHE = None


def _pack_inputs(data_input, mlp_w, mlp_b, CM, attn, cla_w, cla_b):
    x = np.asarray(data_input, dtype=np.float32)
    W = np.asarray(mlp_w, dtype=np.float32)
    b = np.asarray(mlp_b, dtype=np.float32)
    CM = np.asarray(CM, dtype=np.float32)
    attn = np.asarray(attn, dtype=np.float32)
    cla_w = np.asarray(cla_w, dtype=np.float32).reshape(H)
    cla_b = np.asarray(cla_b, dtype=np.float32).reshape(1, 1)

    # Parameter folds (host, O(D*H) — data-independent)
    V = CM @ cla_w                       # [T, H]
    Ga = W.T @ attn                      # [D, T]
    Gs = W.T @ V.T                       # [D, T]
    ca = (b @ attn).reshape(T, 1)
    csp = (G_SCALE * (V @ b)).reshape(T, 1)

    DP = KT * 128
    # x: [B, D] -> per core [128, (g kt j)] fp8, group-major, sizes GBS
    xp = np.zeros((B, DP), dtype=np.float32)
    xp[:, :D] = np.clip(x, -240, 240)
    xp = xp.reshape(NCORES, BLOC, KT, 128)
    slabs = []
    for g in range(NG):
        blk = (xp[:, GOFF[g]:GOFF[g] + GBS[g]]      # [core, gb, KT, 128]
               .transpose(0, 3, 2, 1)               # [core, 128, KT, gb]
               .reshape(NCORES, 128, KT * GBS[g]))
        slabs.append(blk)
    xpk = np.concatenate(slabs, axis=2).astype(NP_F8)
    # G: [D, 2T] -> [128, (kt m)] fp8, scaled, quadrant-padded
    gp = np.zeros((DP, M2), dtype=np.float32)
    gp[:D, 0:T] = np.clip(Ga * G_SCALE, -240, 240)
    gp[:D, 32:32 + T] = np.clip(Gs * G_SCALE, -240, 240)
    gp = (gp.reshape(KT, 128, M2).transpose(1, 0, 2)
            .reshape(128, KT * M2).astype(NP_F8))

    shared = {"gT": gp, "ca": np.ascontiguousarray(ca),
              "csp": np.ascontiguousarray(csp),
              "clabh": np.ascontiguousarray(0.5 * cla_b)}
    return [
        {"xT": np.ascontiguousarray(xpk[i]), **shared}
        for i in range(NCORES)
    ]


def kernel(data_input, mlp_w, mlp_b, CM, attn, cla_w, cla_b):
    global LAST_RESULTS, _NC_CACHE

    in_maps = _pack_inputs(data_input, mlp_w, mlp_b, CM, attn, cla_w, cla_b)

    if _NC_CACHE is None:
        _NC_CACHE = _build_nc()

    trace = bool(int(os.environ.get("KERNEL_TRACE", "0")))
    res = run_bass_kernel_spmd(
        _NC_CACHE, in_maps, core_ids=list(range(NCORES)), trace=trace,
        trace_cores=[0] if trace else None,
    )
    LAST_RESULTS = res

    full = np.empty(B, dtype=np.float32)
    for i in range(NCORES):
        full[i * BLOC:(i + 1) * BLOC] = res.results[i]["out"].reshape(BLOC)
    return full
